# revision 21
# baseline (speedup 1.0000x reference)
"""GAT + global-max-pool + LSTM + Linear kernel for Trainium2 (8 NeuronCores).

Sharding: data-parallel over the batch axis B=8 -> one sequence b per core.
Each core computes the GAT over its 16 graphs (t=0..15), global-max-pools,
runs the LSTM over its sequence locally, and emits one [8] output row.

GAT aggregation (per core, per graph g, head h). The per-dst softmax factor
v[n] = exp(a_d[n]) cancels between numerator and denominator, so with
  u[m] = exp(a_s[m]),  u5[m] = exp(0.2 a_s[m]),  w[n] = exp(-0.8 a_d[n])
the dense (src m, dst n) attention tile reduces to
  tS[m, n] = max(u[m], u5[m] * w[n])          (one DVE tensor_scalar, 2x mode)
  tA[m, n] = tS * cnt[m, n]                   (TT on DVE or gpsimd)
  out[n,f], den[n] via PE matmul with lhsT = [xp_h | ones] (33 rows out)
  gat[n,f] = relu(out/den + b_gat); pooled = max over n.

softmax max-subtraction is dropped: alpha = tA/sum(tA) is invariant to the
per-dst shift and exp of |a| <~ 10 cannot overflow bf16.

Engine budget per (g,h): PE 16 matmuls (oph), DVE 8 tensor_scalar + 5 TT +
epilogue, gpsimd wB/rb broadcasts + 3 TT, ACT copies/exp-fills/orel.
"""

import numpy as np

import concourse.bacc as bacc
import concourse.bass as bass
import concourse.mybir as mybir
import concourse.tile as tile
from concourse.bass_utils import run_bass_kernel_spmd

B, T, N, F_IN = 8, 16, 1000, 16
H, D = 4, 32
HD = H * D          # 128
HL = 64
OUT = 8
NEG = 0.2
NPAD = 1024         # padded node count
NBLK = 8            # src blocks of 128
G = T               # graphs per core
XW = 34             # xp33 stride per (J, h): 32 xp cols + ones col + pad

FP = mybir.dt.float32
BF = mybir.dt.bfloat16
AX = mybir.AxisListType
AF = mybir.ActivationFunctionType
OPS = mybir.AluOpType

_CACHE = {}


def _build_nc():
    nc = bacc.Bacc("TRN2", target_bir_lowering=False, debug=False)

    # ---- DRAM I/O ----
    d_xt = nc.dram_tensor("x_t", [F_IN, G * NPAD], BF, kind="ExternalInput").ap()
    d_cnt = nc.dram_tensor("cntmask", [128, NBLK * NPAD], BF, kind="ExternalInput").ap()
    d_wc = nc.dram_tensor("wc", [F_IN, 132], BF, kind="ExternalInput").ap()
    d_wa8 = nc.dram_tensor("wa8", [F_IN, 128], BF, kind="ExternalInput").ap()
    d_bgat = nc.dram_tensor("b_gat", [32, H], FP, kind="ExternalInput").ap()
    d_bb = nc.dram_tensor("blockones", [H, 128], BF, kind="ExternalInput").ap()
    d_ones1 = nc.dram_tensor("ones1", [1, 128], BF, kind="ExternalInput").ap()
    d_wih = nc.dram_tensor("wih_t", [HD, 4 * HL], FP, kind="ExternalInput").ap()
    d_whh = nc.dram_tensor("whh_t", [HL, 4 * HL], FP, kind="ExternalInput").ap()
    d_bls = nc.dram_tensor("b_lstm", [HL, 4], FP, kind="ExternalInput").ap()
    d_wclf = nc.dram_tensor("wclf_t", [HL, OUT], FP, kind="ExternalInput").ap()
    d_bclf = nc.dram_tensor("b_clf", [OUT, 1], FP, kind="ExternalInput").ap()
    d_y = nc.dram_tensor("y", [OUT, 1], FP, kind="ExternalOutput").ap()

    with tile.TileContext(nc) as tc:
        with (
            tc.tile_pool(name="const", bufs=1) as cpool,
            tc.tile_pool(name="stage", bufs=2) as spool,
            tc.tile_pool(name="wb", bufs=3) as wpool,
            tc.tile_pool(name="edense", bufs=3) as epool,
            tc.tile_pool(name="epi", bufs=3) as mpool,
            tc.tile_pool(name="lstm", bufs=2) as lpool,
        ):
            # ---- load constants ----
            c_xt = cpool.tile([F_IN, G * NPAD], BF, tag="xt")
            nc.sync.dma_start(c_xt[:], d_xt)
            c_cnt = cpool.tile([128, NBLK * NPAD], BF, tag="cnt")
            nc.sync.dma_start(c_cnt[:], d_cnt)
            c_wc = cpool.tile([F_IN, 132], BF, tag="wc")
            nc.sync.dma_start(c_wc[:], d_wc)
            c_wa8 = cpool.tile([F_IN, 128], BF, tag="wa8")
            nc.sync.dma_start(c_wa8[:], d_wa8)
            c_bgat = cpool.tile([32, H], FP, tag="bgat")
            nc.sync.dma_start(c_bgat[:], d_bgat)
            c_bb = cpool.tile([H, 128], BF, tag="bb")
            nc.sync.dma_start(c_bb[:], d_bb)
            c_ones1 = cpool.tile([1, 128], BF, tag="ones1")
            nc.sync.dma_start(c_ones1[:], d_ones1)
            c_wih = cpool.tile([HD, 4 * HL], FP, tag="wih")
            nc.sync.dma_start(c_wih[:], d_wih)
            c_whh = cpool.tile([HL, 4 * HL], FP, tag="whh")
            nc.sync.dma_start(c_whh[:], d_whh)
            c_bls = cpool.tile([HL, 4], FP, tag="bls")
            nc.sync.dma_start(c_bls[:], d_bls)
            c_wclf = cpool.tile([HL, OUT], FP, tag="wclf")
            nc.sync.dma_start(c_wclf[:], d_wclf)
            c_bclf = cpool.tile([OUT, 1], FP, tag="bclf")
            nc.sync.dma_start(c_bclf[:], d_bclf)

            c_pool = cpool.tile([HD, G], FP, tag="pooled")

            gat_pools = (
                tc.tile_pool(name="ps_stage", bufs=2, space="PSUM"),
                tc.tile_pool(name="ps_out", bufs=1, space="PSUM"),
                tc.tile_pool(name="ps_rb", bufs=1, space="PSUM"),
                tc.tile_pool(name="ps_wb", bufs=1, space="PSUM"),
            )
            ps_stage = gat_pools[0].__enter__()
            ps_out = gat_pools[1].__enter__()
            ps_rb = gat_pools[2].__enter__()
            ps_wb = gat_pools[3].__enter__()
            def issue_epilogue(eg, st):
                # batched per-graph epilogue: one reciprocal for all 4 heads,
                # PE block-broadcast of the reciprocal rows, then per-head
                # divide + bias + relu into porel.
                rec4 = mpool.tile([32, 128], BF, tag="rec4")
                with nc.allow_low_precision(reason="bf16 reciprocal: 0.4% rel err ok"):
                    nc.vector.reciprocal(rec4[:], st["den4"][:])
                rech4 = mpool.tile([H, NPAD], BF, tag="rech4")
                for h in range(H):
                    nc.sync.dma_start(
                        rech4[h:h + 1, :],
                        rec4[:].rearrange("i (h j) -> i h j", j=32)[:, h, :],
                    )
                rb = ps_rb.tile([128, NPAD], FP, tag="rb")
                for half in range(2):
                    nc.tensor.matmul(
                        rb[:, half * 512:(half + 1) * 512], c_bb[:],
                        rech4[:, half * 512:(half + 1) * 512],
                        start=True, stop=True,
                    )
                for h in range(H):
                    rbS = mpool.tile([32, NPAD], BF, tag=f"rbS{h}")
                    nc.scalar.copy(rbS[:], rb[32 * h:32 * (h + 1), :])
                    odiv = mpool.tile([32, NPAD], BF, tag="odiv")
                    nc.vector.tensor_tensor(
                        odiv[:], st["ophS"][h][0:32, :], rbS[:], OPS.mult,
                    )
                    nc.scalar.activation(
                        st["porel"][32 * h:32 * (h + 1), :], odiv[:], AF.Relu,
                        bias=c_bgat[:, h:h + 1], scale=1.0,
                    )

            def issue_reduce(eg, st):
                nc.vector.tensor_reduce(
                    c_pool[:, eg:eg + 1], st["porel"][:, 0:N], AX.X, OPS.max
                )

            def issue_stage(g):
                goff = g * NPAD
                xp33 = spool.tile([128, NBLK * H * XW], BF, tag="xp33")
                nc.vector.memset(
                    xp33[:].rearrange("p (J h q) -> p J h q", h=H, q=XW)[
                        :, :, :, 32:33
                    ],
                    1.0,
                )
                u_t = spool.tile([128, NBLK * H], FP, tag="u_t")
                u5_t = spool.tile([128, NBLK * H], FP, tag="u5_t")
                for J in range(NBLK):
                    pS = ps_stage.tile([128, 512], FP, tag="st")
                    pS = pS[:, 0:132]
                    nc.tensor.matmul(
                        pS[:], c_xt[:, goff + J * 128:goff + (J + 1) * 128],
                        c_wc[:], start=True, stop=True,
                    )
                    nc.scalar.copy(
                        xp33[:, J * H * XW:(J + 1) * H * XW].rearrange(
                            "p (h q) -> p h q", q=XW
                        )[:, :, 0:32],
                        pS[:, 0:128].rearrange("p (h q) -> p h q", q=32),
                    )
                    nc.scalar.activation(
                        u_t[:, J * H:(J + 1) * H], pS[:, 128:132], AF.Exp,
                        scale=1.0,
                    )
                    nc.scalar.activation(
                        u5_t[:, J * H:(J + 1) * H], pS[:, 128:132], AF.Exp,
                        scale=NEG,
                    )
                # w rows: exp(-0.8 a_d) at partitions 32h of s8w
                s8w = spool.tile([128, NPAD], BF, tag="s8w")
                for half in range(2):
                    pw = ps_stage.tile([128, 512], FP, tag="st")
                    nc.tensor.matmul(
                        pw[:], c_wa8[:],
                        c_xt[:, goff + half * 512:goff + (half + 1) * 512],
                        start=True, stop=True,
                    )
                    nc.scalar.activation(
                        s8w[:, half * 512:(half + 1) * 512], pw[:], AF.Exp,
                        scale=NEG - 1.0,
                    )
                # relocate w rows to partition 0 (PE lhsT/rhs base rule)
                wrs = []
                for h in range(H):
                    wr = wpool.tile([1, NPAD], BF, tag=f"wrow{h}")
                    nc.sync.dma_start(wr[:], s8w[32 * h:32 * h + 1, :])
                    wrs.append(wr)
                porel = spool.tile([128, NPAD], BF, tag="porel")
                den4 = mpool.tile([32, 128], BF, tag="den4")
                return {"xp33": xp33, "u_t": u_t, "u5_t": u5_t, "wrs": wrs,
                        "porel": porel, "den4": den4, "ophS": [], "oph": None}

            def issue_wb(st, h):
                # broadcast w row to 128 partitions: PE ones-matmul into
                # PSUM, then ACT copy to SBUF bf16
                wbp = ps_wb.tile([128, NPAD], FP, tag="wbp")
                for half in range(2):
                    nc.tensor.matmul(
                        wbp[:, half * 512:(half + 1) * 512], c_ones1[:],
                        st["wrs"][h][:, half * 512:(half + 1) * 512],
                        start=True, stop=True,
                    )
                wB = wpool.tile([128, NPAD], BF, tag="wB")
                nc.scalar.copy(wB[:], wbp[:])
                return wB

            def issue_dense_head(st, h, wB):
                if h % 2 == 0:
                    ophDB = ps_out.tile([128, NPAD], FP, tag="oph")
                    st["oph"] = ophDB
                oph = st["oph"][64 * (h % 2):64 * (h % 2) + 33]
                u_t, u5_t, xp33 = st["u_t"], st["u5_t"], st["xp33"]
                for J in range(NBLK):
                    col = J * H + h
                    tS = epool.tile([128, NPAD], BF, tag="tS")
                    nc.vector.tensor_scalar(
                        tS[:], wB[:], u5_t[:, col:col + 1],
                        u_t[:, col:col + 1], OPS.mult, OPS.max,
                    )
                    tA = epool.tile([128, NPAD], BF, tag="tA")
                    nc.vector.tensor_tensor(
                        tA[:], tS[:], c_cnt[:, J * NPAD:(J + 1) * NPAD],
                        OPS.mult,
                    )
                    base = J * H * XW + h * XW
                    for half in range(2):
                        nc.tensor.matmul(
                            oph[:, half * 512:(half + 1) * 512],
                            xp33[:, base:base + 33],
                            tA[:, half * 512:(half + 1) * 512],
                            start=(J == 0), stop=(J == NBLK - 1),
                        )
                # copy PSUM out + stash den row
                ophS = mpool.tile([33, NPAD], BF, tag=f"ophS{h}")
                nc.scalar.copy(ophS[:], oph[:])
                nc.sync.dma_start(
                    st["den4"][:, 32 * h:32 * (h + 1)], ophS[32:33, :]
                )
                st["ophS"].append(ophS)

            # LSTM state: h stored as h2 = 2h (W_hh/W_clf pre-halved on
            # host); c stored as c2 = 2c (tanh applied with scale=0.5).
            lstm_h0 = lpool.tile([HL, 1], FP, tag="h0")
            lstm_c0 = lpool.tile([HL, 1], FP, tag="c0")
            lstate = {"h": lstm_h0, "c": lstm_c0}
            nc.vector.memset(lstate["h"][:], 0.0)
            nc.vector.memset(lstate["c"][:], 0.0)

            def issue_lstm_step(t):
                hprev, cprev = lstate["h"], lstate["c"]
                tga = []
                for gate in range(4):
                    psg = ps_stage.tile([128, 512], FP, tag="st")
                    psg = psg[0:HL, 0:1]
                    nc.tensor.matmul(
                        psg, c_wih[:, gate * HL:(gate + 1) * HL],
                        c_pool[:, t:t + 1], start=True, stop=False,
                    )
                    nc.tensor.matmul(
                        psg, c_whh[:, gate * HL:(gate + 1) * HL],
                        hprev[:], start=False, stop=True,
                    )
                    tgt = lpool.tile([HL, 1], FP, tag=f"tg{gate}")
                    # gates i,f,o: sigmoid via tanh-half; gate g: plain tanh
                    sc = 1.0 if gate == 2 else 0.5
                    nc.scalar.activation(
                        tgt[:], psg, AF.Tanh,
                        bias=c_bls[:, gate:gate + 1], scale=sc,
                    )
                    tga.append(tgt)
                ti, tf, tg_, to = tga
                # v1 = (tf+1)*c2 = 4*sig(f)*c ; v2 = (ti+1)*tg = 2*sig(i)*g
                v1 = lpool.tile([HL, 1], FP, tag="v1")
                nc.vector.scalar_tensor_tensor(
                    v1[:], tf[:], 1.0, cprev[:], OPS.add, OPS.mult
                )
                v2 = lpool.tile([HL, 1], FP, tag="v2")
                nc.vector.scalar_tensor_tensor(
                    v2[:], ti[:], 1.0, tg_[:], OPS.add, OPS.mult
                )
                cnew = lpool.tile([HL, 1], FP, tag="c0")
                nc.vector.scalar_tensor_tensor(
                    cnew[:], v1[:], 0.5, v2[:], OPS.mult, OPS.add
                )
                tcn = lpool.tile([HL, 1], FP, tag="tcn")
                nc.scalar.activation(tcn[:], cnew[:], AF.Tanh, scale=0.5)
                hnew = lpool.tile([HL, 1], FP, tag="h0")
                nc.vector.scalar_tensor_tensor(
                    hnew[:], to[:], 1.0, tcn[:], OPS.add, OPS.mult
                )
                lstate["h"], lstate["c"] = hnew, cnew

            stages = {0: issue_stage(0)}
            cur_wb = issue_wb(stages[0], 0)
            for g in range(G):
                st = stages[g]
                for h in range(H):
                    if h < H - 1:
                        upcoming = issue_wb(st, h + 1)
                    elif g < G - 1:
                        upcoming = issue_wb(stages[g + 1], 0)
                    else:
                        upcoming = None
                    issue_dense_head(st, h, cur_wb)
                    if h == 0 and g > 0:
                        issue_epilogue(g - 1, stages[g - 1])
                    if h == 1 and g > 0:
                        issue_reduce(g - 1, stages[g - 1])
                        issue_lstm_step(g - 1)
                        del stages[g - 1]
                    if h == 2 and g < G - 1:
                        stages[g + 1] = issue_stage(g + 1)
                    cur_wb = upcoming
            issue_epilogue(G - 1, stages[G - 1])
            issue_reduce(G - 1, stages[G - 1])
            issue_lstm_step(G - 1)

            ps3t = ps_stage.tile([128, 512], FP, tag="st")
            ps3 = ps3t[0:OUT, 0:1]
            nc.tensor.matmul(ps3, c_wclf[:], lstate["h"][:], start=True, stop=True)
            ysb = lpool.tile([OUT, 1], FP, tag="ysb")
            nc.vector.tensor_tensor(ysb[:], ps3, c_bclf[:], OPS.add)
            nc.sync.dma_start(d_y, ysb[:])

            gat_pools[3].__exit__(None, None, None)
            gat_pools[2].__exit__(None, None, None)
            gat_pools[1].__exit__(None, None, None)
            gat_pools[0].__exit__(None, None, None)

    nc.compile()
    return nc


def _host_prep(inputs):
    x = np.asarray(inputs["x"], dtype=np.float32)          # [B, T, N, F]
    ei = np.asarray(inputs["edge_index"])
    W_gat = np.asarray(inputs["W_gat"], dtype=np.float32)  # [16, 128]
    att_src = np.asarray(inputs["att_src"], dtype=np.float32)  # [H, D]
    att_dst = np.asarray(inputs["att_dst"], dtype=np.float32)
    b_gat = np.asarray(inputs["b_gat"], dtype=np.float32)
    W_ih = np.asarray(inputs["W_ih"], dtype=np.float32)    # [256, 128]
    W_hh = np.asarray(inputs["W_hh"], dtype=np.float32)    # [256, 64]
    b_ih = np.asarray(inputs["b_ih"], dtype=np.float32)
    b_hh = np.asarray(inputs["b_hh"], dtype=np.float32)
    W_clf = np.asarray(inputs["W_clf"], dtype=np.float32)  # [8, 64]
    b_clf = np.asarray(inputs["b_clf"], dtype=np.float32)

    bf16 = mybir.dt.np(BF)

    # fold attention vectors: a_s = x @ (W_gat-reshaped @ att_src)
    Wr = W_gat.reshape(F_IN, H, D)
    W_as = np.einsum("fhd,hd->fh", Wr, att_src)            # [16, 4]
    W_ad = np.einsum("fhd,hd->fh", Wr, att_dst)
    wc = np.zeros((F_IN, 132), dtype=np.float32)
    wc[:, 0:128] = W_gat
    wc[:, 128:132] = W_as
    wa8 = np.zeros((F_IN, 128), dtype=np.float32)
    wa8[:, 32 * np.arange(H)] = W_ad                       # a_d -> partition 32h

    # edge counts with self loops, dense [1024, 1024]
    src = ei[0].astype(np.int64)
    dst = ei[1].astype(np.int64)
    Cm = np.zeros((NPAD, NPAD), dtype=np.float32)
    np.add.at(Cm, (src, dst), 1.0)
    Cm[np.arange(N), np.arange(N)] += 1.0                  # self loops
    Cm[NPAD - 1, N:] = 1.0  # dummy edges: keep pad-column denominators finite
    cntmask = (
        Cm.reshape(NBLK, 128, NPAD).transpose(1, 0, 2).reshape(128, NBLK * NPAD)
    ).astype(bf16)

    # x transposed per core: [F, T*NPAD] bf16
    xpad = np.zeros((B, T, NPAD, F_IN), dtype=np.float32)
    xpad[:, :, :N, :] = x
    xts = [
        np.ascontiguousarray(xpad[b].reshape(T * NPAD, F_IN).T).astype(bf16)
        for b in range(B)
    ]

    b_gates = (b_ih + b_hh).astype(np.float32)             # [256]
    bls = np.zeros((HL, 4), dtype=np.float32)
    bls[:, 0] = 0.5 * b_gates[0:64]                        # i (tanh-half trick)
    bls[:, 1] = 0.5 * b_gates[64:128]                      # f
    bls[:, 2] = b_gates[128:192]                           # g
    bls[:, 3] = 0.5 * b_gates[192:256]                     # o

    bb = np.zeros((H, 128), dtype=np.float32)
    for h in range(H):
        bb[h, 32 * h:32 * (h + 1)] = 1.0

    common = {
        "blockones": bb.astype(bf16),
        "ones1": np.ones((1, 128), dtype=bf16),
        "cntmask": cntmask,
        "wc": wc.astype(bf16),
        "wa8": wa8.astype(bf16),
        "b_gat": np.ascontiguousarray(b_gat.reshape(H, 32).T),
        "wih_t": np.ascontiguousarray(W_ih.T),             # [128, 256]
        "whh_t": np.ascontiguousarray(0.5 * W_hh.T),       # [64, 256] (h2 comp)
        "b_lstm": bls,
        "wclf_t": np.ascontiguousarray(0.5 * W_clf.T),     # [64, 8] (h2 comp)
        "b_clf": b_clf.reshape(OUT, 1),
    }
    in_maps = []
    for b in range(B):
        m = dict(common)
        m["x_t"] = xts[b]
        in_maps.append(m)
    return in_maps


def kernel(**inputs):
    if "nc" not in _CACHE:
        _CACHE["nc"] = _build_nc()
    nc = _CACHE["nc"]
    in_maps = _host_prep(inputs)
    res = run_bass_kernel_spmd(nc, in_maps, core_ids=list(range(B)))
    y = np.stack([r["y"][:, 0] for r in res.results], axis=0)
    return y.astype(np.float32)


if __name__ == "__main__":
    import reference as R

    inp = R.setup_inputs()
    inp = {k: np.asarray(v) for k, v in inp.items()}
    out = kernel(**inp)
    print(out)


# revision 22
# speedup vs baseline: 1.0347x; 1.0347x over previous
"""GAT + global-max-pool + LSTM + Linear kernel for Trainium2 (8 NeuronCores).

Sharding: data-parallel over the batch axis B=8 -> one sequence b per core.
Each core computes the GAT over its 16 graphs (t=0..15), global-max-pools,
runs the LSTM over its sequence locally, and emits one [8] output row.

GAT aggregation (per core, per graph g, head h). The per-dst softmax factor
v[n] = exp(a_d[n]) cancels between numerator and denominator, so with
  u[m] = exp(a_s[m]),  u5[m] = exp(0.2 a_s[m]),  w[n] = exp(-0.8 a_d[n])
the dense (src m, dst n) attention tile reduces to
  tS[m, n] = max(u[m], u5[m] * w[n])          (one DVE tensor_scalar, 2x mode)
  tA[m, n] = tS * cnt[m, n]                   (TT on DVE or gpsimd)
  out[n,f], den[n] via PE matmul with lhsT = [xp_h | ones] (33 rows out)
  gat[n,f] = relu(out/den + b_gat); pooled = max over n.

softmax max-subtraction is dropped: alpha = tA/sum(tA) is invariant to the
per-dst shift and exp of |a| <~ 10 cannot overflow bf16.

Engine budget per (g,h): PE 16 matmuls (oph), DVE 8 tensor_scalar + 5 TT +
epilogue, gpsimd wB/rb broadcasts + 3 TT, ACT copies/exp-fills/orel.
"""

import numpy as np

import concourse.bacc as bacc
import concourse.bass as bass
import concourse.mybir as mybir
import concourse.tile as tile
from concourse.bass_utils import run_bass_kernel_spmd

B, T, N, F_IN = 8, 16, 1000, 16
H, D = 4, 32
HD = H * D          # 128
HL = 64
OUT = 8
NEG = 0.2
NPAD = 1024         # padded node count
NBLK = 8            # src blocks of 128
G = T               # graphs per core
XW = 34             # xp33 stride per (J, h): 32 xp cols + ones col + pad

FP = mybir.dt.float32
BF = mybir.dt.bfloat16
AX = mybir.AxisListType
AF = mybir.ActivationFunctionType
OPS = mybir.AluOpType

_CACHE = {}


def _build_nc():
    nc = bacc.Bacc("TRN2", target_bir_lowering=False, debug=False)

    # ---- DRAM I/O ----
    d_xt = nc.dram_tensor("x_t", [F_IN, G * NPAD], BF, kind="ExternalInput").ap()
    d_cnt = nc.dram_tensor("cntmask", [128, NBLK * NPAD], BF, kind="ExternalInput").ap()
    d_wc = nc.dram_tensor("wc", [F_IN, 132], BF, kind="ExternalInput").ap()
    d_wa8 = nc.dram_tensor("wa8", [F_IN, 128], BF, kind="ExternalInput").ap()
    d_bgat = nc.dram_tensor("b_gat", [32, H], FP, kind="ExternalInput").ap()
    d_bb = nc.dram_tensor("blockones", [H, 128], BF, kind="ExternalInput").ap()
    d_ones1 = nc.dram_tensor("ones1", [1, 128], BF, kind="ExternalInput").ap()
    d_wih = nc.dram_tensor("wih_t", [HD, 4 * HL], FP, kind="ExternalInput").ap()
    d_whh = nc.dram_tensor("whh_t", [HL, 4 * HL], FP, kind="ExternalInput").ap()
    d_bls = nc.dram_tensor("b_lstm", [HL, 4], FP, kind="ExternalInput").ap()
    d_wclf = nc.dram_tensor("wclf_t", [HL, OUT], FP, kind="ExternalInput").ap()
    d_bclf = nc.dram_tensor("b_clf", [OUT, 1], FP, kind="ExternalInput").ap()
    d_y = nc.dram_tensor("y", [OUT, 1], FP, kind="ExternalOutput").ap()

    with tile.TileContext(nc) as tc:
        with (
            tc.tile_pool(name="const", bufs=1) as cpool,
            tc.tile_pool(name="stage", bufs=2) as spool,
            tc.tile_pool(name="wb", bufs=3) as wpool,
            tc.tile_pool(name="edense", bufs=3) as epool,
            tc.tile_pool(name="epi", bufs=3) as mpool,
            tc.tile_pool(name="lstm", bufs=2) as lpool,
        ):
            # ---- load constants ----
            c_xt = cpool.tile([F_IN, G * NPAD], BF, tag="xt")
            nc.sync.dma_start(c_xt[:], d_xt)
            c_cnt = cpool.tile([128, NBLK * NPAD], BF, tag="cnt")
            nc.sync.dma_start(c_cnt[:], d_cnt)
            c_wc = cpool.tile([F_IN, 132], BF, tag="wc")
            nc.sync.dma_start(c_wc[:], d_wc)
            c_wa8 = cpool.tile([F_IN, 128], BF, tag="wa8")
            nc.sync.dma_start(c_wa8[:], d_wa8)
            c_bgat = cpool.tile([32, H], FP, tag="bgat")
            nc.sync.dma_start(c_bgat[:], d_bgat)
            c_bb = cpool.tile([H, 128], BF, tag="bb")
            nc.sync.dma_start(c_bb[:], d_bb)
            c_ones1 = cpool.tile([1, 128], BF, tag="ones1")
            nc.sync.dma_start(c_ones1[:], d_ones1)
            c_wih = cpool.tile([HD, 4 * HL], FP, tag="wih")
            nc.sync.dma_start(c_wih[:], d_wih)
            c_whh = cpool.tile([HL, 4 * HL], FP, tag="whh")
            nc.sync.dma_start(c_whh[:], d_whh)
            c_bls = cpool.tile([HL, 4], FP, tag="bls")
            nc.sync.dma_start(c_bls[:], d_bls)
            c_wclf = cpool.tile([HL, OUT], FP, tag="wclf")
            nc.sync.dma_start(c_wclf[:], d_wclf)
            c_bclf = cpool.tile([OUT, 1], FP, tag="bclf")
            nc.sync.dma_start(c_bclf[:], d_bclf)

            c_pool = cpool.tile([HD, G], FP, tag="pooled")

            gat_pools = (
                tc.tile_pool(name="ps_stage", bufs=2, space="PSUM"),
                tc.tile_pool(name="ps_out", bufs=1, space="PSUM"),
                tc.tile_pool(name="ps_rb", bufs=1, space="PSUM"),
                tc.tile_pool(name="ps_wb", bufs=1, space="PSUM"),
            )
            ps_stage = gat_pools[0].__enter__()
            ps_out = gat_pools[1].__enter__()
            ps_rb = gat_pools[2].__enter__()
            ps_wb = gat_pools[3].__enter__()
            def issue_epilogue(eg, st):
                # batched per-graph epilogue: one reciprocal for all 4 heads,
                # PE block-broadcast of the reciprocal rows, then per-head
                # divide + bias + relu into porel.
                rec4 = mpool.tile([32, 128], BF, tag="rec4")
                with nc.allow_low_precision(reason="bf16 reciprocal: 0.4% rel err ok"):
                    nc.vector.reciprocal(rec4[:], st["den4"][:])
                rech4 = mpool.tile([H, NPAD], BF, tag="rech4")
                for h in range(H):
                    nc.sync.dma_start(
                        rech4[h:h + 1, :],
                        rec4[:].rearrange("i (h j) -> i h j", j=32)[:, h, :],
                    )
                rb = ps_rb.tile([128, NPAD], FP, tag="rb")
                for half in range(2):
                    nc.tensor.matmul(
                        rb[:, half * 512:(half + 1) * 512], c_bb[:],
                        rech4[:, half * 512:(half + 1) * 512],
                        start=True, stop=True,
                    )
                for h in range(H):
                    rbS = mpool.tile([32, NPAD], BF, tag=f"rbS{h}")
                    nc.scalar.copy(rbS[:], rb[32 * h:32 * (h + 1), :])
                    odiv = mpool.tile([32, NPAD], BF, tag="odiv")
                    nc.vector.tensor_tensor(
                        odiv[:], st["ophS"][h][0:32, :], rbS[:], OPS.mult,
                    )
                    nc.scalar.activation(
                        st["porel"][32 * h:32 * (h + 1), :], odiv[:], AF.Relu,
                        bias=c_bgat[:, h:h + 1], scale=1.0,
                    )

            def issue_reduce(eg, st):
                nc.vector.tensor_reduce(
                    c_pool[:, eg:eg + 1], st["porel"][:, 0:N], AX.X, OPS.max
                )

            # LSTM state: h stored as h2 = 2h (W_hh/W_clf pre-halved on
            # host); c stored as c2 = 2c (tanh applied with scale=0.5).
            lstm_h0 = lpool.tile([HL, 1], FP, tag="h0")
            lstm_c0 = lpool.tile([HL, 1], FP, tag="c0")
            lstate = {"h": lstm_h0, "c": lstm_c0}
            nc.vector.memset(lstate["h"][:], 0.0)
            nc.vector.memset(lstate["c"][:], 0.0)

            def issue_lstm_step(t):
                hprev, cprev = lstate["h"], lstate["c"]
                tga = []
                for gate in range(4):
                    psgt = ps_stage.tile([128, 512], FP, tag="st")
                    psg = psgt[0:HL, 0:1]
                    nc.tensor.matmul(
                        psg, c_wih[:, gate * HL:(gate + 1) * HL],
                        c_pool[:, t:t + 1], start=True, stop=False,
                    )
                    nc.tensor.matmul(
                        psg, c_whh[:, gate * HL:(gate + 1) * HL],
                        hprev[:], start=False, stop=True,
                    )
                    tgt = lpool.tile([HL, 1], FP, tag=f"tg{gate}")
                    # gates i,f,o: sigmoid via tanh-half; gate g: plain tanh
                    sc = 1.0 if gate == 2 else 0.5
                    nc.scalar.activation(
                        tgt[:], psg, AF.Tanh,
                        bias=c_bls[:, gate:gate + 1], scale=sc,
                    )
                    tga.append(tgt)
                ti, tf, tg_, to = tga
                # v1 = (tf+1)*c2 = 4*sig(f)*c ; v2 = (ti+1)*tg = 2*sig(i)*g
                v1 = lpool.tile([HL, 1], FP, tag="v1")
                nc.vector.scalar_tensor_tensor(
                    v1[:], tf[:], 1.0, cprev[:], OPS.add, OPS.mult
                )
                v2 = lpool.tile([HL, 1], FP, tag="v2")
                nc.vector.scalar_tensor_tensor(
                    v2[:], ti[:], 1.0, tg_[:], OPS.add, OPS.mult
                )
                cnew = lpool.tile([HL, 1], FP, tag="c0")
                nc.vector.scalar_tensor_tensor(
                    cnew[:], v1[:], 0.5, v2[:], OPS.mult, OPS.add
                )
                tcn = lpool.tile([HL, 1], FP, tag="tcn")
                nc.scalar.activation(tcn[:], cnew[:], AF.Tanh, scale=0.5)
                hnew = lpool.tile([HL, 1], FP, tag="h0")
                nc.vector.scalar_tensor_tensor(
                    hnew[:], to[:], 1.0, tcn[:], OPS.add, OPS.mult
                )
                lstate["h"], lstate["c"] = hnew, cnew

            prev = None
            for g in range(G):
                goff = g * NPAD
                # ---- stage: per-J combined matmul -> xp (m-partitioned) + a_s
                xp33 = spool.tile([128, NBLK * H * XW], BF, tag="xp33")
                nc.vector.memset(
                    xp33[:].rearrange("p (J h q) -> p J h q", h=H, q=XW)[
                        :, :, :, 32:33
                    ],
                    1.0,
                )
                u_t = spool.tile([128, NBLK * H], FP, tag="u_t")
                u5_t = spool.tile([128, NBLK * H], FP, tag="u5_t")
                for J in range(NBLK):
                    pS = ps_stage.tile([128, 512], FP, tag="st")
                    pS = pS[:, 0:132]
                    nc.tensor.matmul(
                        pS[:], c_xt[:, goff + J * 128:goff + (J + 1) * 128],
                        c_wc[:], start=True, stop=True,
                    )
                    # xp -> xp33 strided slots (ACT copy, bf16)
                    nc.scalar.copy(
                        xp33[:, J * H * XW:(J + 1) * H * XW].rearrange(
                            "p (h q) -> p h q", q=XW
                        )[:, :, 0:32],
                        pS[:, 0:128].rearrange("p (h q) -> p h q", q=32),
                    )
                    nc.scalar.activation(
                        u_t[:, J * H:(J + 1) * H], pS[:, 128:132], AF.Exp,
                        scale=1.0,
                    )
                    nc.scalar.activation(
                        u5_t[:, J * H:(J + 1) * H], pS[:, 128:132], AF.Exp,
                        scale=NEG,
                    )
                # w rows: exp(-0.8 a_d) at partitions 32h of s8w
                s8w = spool.tile([128, NPAD], BF, tag="s8w")
                for half in range(2):
                    pw = ps_stage.tile([128, 512], FP, tag="st")
                    nc.tensor.matmul(
                        pw[:], c_wa8[:],
                        c_xt[:, goff + half * 512:goff + (half + 1) * 512],
                        start=True, stop=True,
                    )
                    nc.scalar.activation(
                        s8w[:, half * 512:(half + 1) * 512], pw[:], AF.Exp,
                        scale=NEG - 1.0,
                    )

                porel = spool.tile([128, NPAD], BF, tag="porel")
                den4 = mpool.tile([32, 128], BF, tag="den4")
                cur = {"den4": den4, "porel": porel, "ophS": []}

                # partition_broadcast only reads physical partition 0: relocate
                # all four w rows there at stage time (keeps these DMAs clear
                # of the den4-fold DMAs in the queue)
                wrs = []
                for h in range(H):
                    wr = wpool.tile([1, NPAD], BF, tag=f"wrow{h}")
                    nc.sync.dma_start(wr[:], s8w[32 * h:32 * h + 1, :])
                    wrs.append(wr)

                def issue_wb(h):
                    # broadcast w row to 128 partitions: PE ones-matmul into
                    # PSUM, then ACT copy to SBUF bf16 (gpsimd stays pure-TT:
                    # op-type switches cost ~6us reconfig)
                    wbp = ps_wb.tile([128, NPAD], FP, tag="wbp")
                    for half in range(2):
                        nc.tensor.matmul(
                            wbp[:, half * 512:(half + 1) * 512], c_ones1[:],
                            wrs[h][:, half * 512:(half + 1) * 512],
                            start=True, stop=True,
                        )
                    wB = wpool.tile([128, NPAD], BF, tag="wB")
                    nc.scalar.copy(wB[:], wbp[:])
                    return wB

                wbs = [issue_wb(0)]
                for h in range(H):
                    if h == 1 and prev is not None:
                        issue_epilogue(g - 1, prev)
                    if h == 2 and prev is not None:
                        issue_reduce(g - 1, prev)
                        issue_lstm_step(g - 1)
                    if h < H - 1:
                        wbs.append(issue_wb(h + 1))
                    wB = wbs[h]
                    if h % 2 == 0:
                        ophDB = ps_out.tile([128, NPAD], FP, tag="oph")
                    oph = ophDB[64 * (h % 2):64 * (h % 2) + 33]
                    for J in range(NBLK):
                        col = J * H + h
                        tS = epool.tile([128, NPAD], BF, tag="tS")
                        nc.vector.tensor_scalar(
                            tS[:], wB[:], u5_t[:, col:col + 1],
                            u_t[:, col:col + 1], OPS.mult, OPS.max,
                        )
                        tA = epool.tile([128, NPAD], BF, tag="tA")
                        eng = nc.vector
                        eng.tensor_tensor(
                            tA[:], tS[:], c_cnt[:, J * NPAD:(J + 1) * NPAD],
                            OPS.mult,
                        )
                        base = J * H * XW + h * XW
                        for half in range(2):
                            nc.tensor.matmul(
                                oph[:, half * 512:(half + 1) * 512],
                                xp33[:, base:base + 33],
                                tA[:, half * 512:(half + 1) * 512],
                                start=(J == 0), stop=(J == NBLK - 1),
                            )
                    # ---- per-head: copy PSUM out + stash den row ----
                    ophS = mpool.tile([33, NPAD], BF, tag=f"ophS{h}")
                    nc.scalar.copy(ophS[:], oph[:])
                    nc.sync.dma_start(
                        den4[:, 32 * h:32 * (h + 1)], ophS[32:33, :]
                    )
                    cur["ophS"].append(ophS)
                prev = cur
            issue_epilogue(G - 1, prev)
            issue_reduce(G - 1, prev)
            issue_lstm_step(G - 1)

            ps3t = ps_stage.tile([128, 512], FP, tag="st")
            ps3 = ps3t[0:OUT, 0:1]
            nc.tensor.matmul(ps3, c_wclf[:], lstate["h"][:], start=True, stop=True)
            ysb = lpool.tile([OUT, 1], FP, tag="ysb")
            nc.vector.tensor_tensor(ysb[:], ps3, c_bclf[:], OPS.add)
            nc.sync.dma_start(d_y, ysb[:])

            gat_pools[3].__exit__(None, None, None)
            gat_pools[2].__exit__(None, None, None)
            gat_pools[1].__exit__(None, None, None)
            gat_pools[0].__exit__(None, None, None)

    nc.compile()
    return nc


def _host_prep(inputs):
    x = np.asarray(inputs["x"], dtype=np.float32)          # [B, T, N, F]
    ei = np.asarray(inputs["edge_index"])
    W_gat = np.asarray(inputs["W_gat"], dtype=np.float32)  # [16, 128]
    att_src = np.asarray(inputs["att_src"], dtype=np.float32)  # [H, D]
    att_dst = np.asarray(inputs["att_dst"], dtype=np.float32)
    b_gat = np.asarray(inputs["b_gat"], dtype=np.float32)
    W_ih = np.asarray(inputs["W_ih"], dtype=np.float32)    # [256, 128]
    W_hh = np.asarray(inputs["W_hh"], dtype=np.float32)    # [256, 64]
    b_ih = np.asarray(inputs["b_ih"], dtype=np.float32)
    b_hh = np.asarray(inputs["b_hh"], dtype=np.float32)
    W_clf = np.asarray(inputs["W_clf"], dtype=np.float32)  # [8, 64]
    b_clf = np.asarray(inputs["b_clf"], dtype=np.float32)

    bf16 = mybir.dt.np(BF)

    # fold attention vectors: a_s = x @ (W_gat-reshaped @ att_src)
    Wr = W_gat.reshape(F_IN, H, D)
    W_as = np.einsum("fhd,hd->fh", Wr, att_src)            # [16, 4]
    W_ad = np.einsum("fhd,hd->fh", Wr, att_dst)
    wc = np.zeros((F_IN, 132), dtype=np.float32)
    wc[:, 0:128] = W_gat
    wc[:, 128:132] = W_as
    wa8 = np.zeros((F_IN, 128), dtype=np.float32)
    wa8[:, 32 * np.arange(H)] = W_ad                       # a_d -> partition 32h

    # edge counts with self loops, dense [1024, 1024]
    src = ei[0].astype(np.int64)
    dst = ei[1].astype(np.int64)
    Cm = np.zeros((NPAD, NPAD), dtype=np.float32)
    np.add.at(Cm, (src, dst), 1.0)
    Cm[np.arange(N), np.arange(N)] += 1.0                  # self loops
    Cm[NPAD - 1, N:] = 1.0  # dummy edges: keep pad-column denominators finite
    cntmask = (
        Cm.reshape(NBLK, 128, NPAD).transpose(1, 0, 2).reshape(128, NBLK * NPAD)
    ).astype(bf16)

    # x transposed per core: [F, T*NPAD] bf16
    xpad = np.zeros((B, T, NPAD, F_IN), dtype=np.float32)
    xpad[:, :, :N, :] = x
    xts = [
        np.ascontiguousarray(xpad[b].reshape(T * NPAD, F_IN).T).astype(bf16)
        for b in range(B)
    ]

    b_gates = (b_ih + b_hh).astype(np.float32)             # [256]
    bls = np.zeros((HL, 4), dtype=np.float32)
    bls[:, 0] = 0.5 * b_gates[0:64]                        # i (tanh-half trick)
    bls[:, 1] = 0.5 * b_gates[64:128]                      # f
    bls[:, 2] = b_gates[128:192]                           # g
    bls[:, 3] = 0.5 * b_gates[192:256]                     # o

    bb = np.zeros((H, 128), dtype=np.float32)
    for h in range(H):
        bb[h, 32 * h:32 * (h + 1)] = 1.0

    common = {
        "blockones": bb.astype(bf16),
        "ones1": np.ones((1, 128), dtype=bf16),
        "cntmask": cntmask,
        "wc": wc.astype(bf16),
        "wa8": wa8.astype(bf16),
        "b_gat": np.ascontiguousarray(b_gat.reshape(H, 32).T),
        "wih_t": np.ascontiguousarray(W_ih.T),             # [128, 256]
        "whh_t": np.ascontiguousarray(0.5 * W_hh.T),       # [64, 256] (h2 comp)
        "b_lstm": bls,
        "wclf_t": np.ascontiguousarray(0.5 * W_clf.T),     # [64, 8] (h2 comp)
        "b_clf": b_clf.reshape(OUT, 1),
    }
    in_maps = []
    for b in range(B):
        m = dict(common)
        m["x_t"] = xts[b]
        in_maps.append(m)
    return in_maps


def kernel(**inputs):
    if "nc" not in _CACHE:
        _CACHE["nc"] = _build_nc()
    nc = _CACHE["nc"]
    in_maps = _host_prep(inputs)
    res = run_bass_kernel_spmd(nc, in_maps, core_ids=list(range(B)))
    y = np.stack([r["y"][:, 0] for r in res.results], axis=0)
    return y.astype(np.float32)


if __name__ == "__main__":
    import reference as R

    inp = R.setup_inputs()
    inp = {k: np.asarray(v) for k, v in inp.items()}
    out = kernel(**inp)
    print(out)


# revision 23
# speedup vs baseline: 1.1149x; 1.0775x over previous
"""GAT + global-max-pool + LSTM + Linear kernel for Trainium2 (8 NeuronCores).

Sharding: data-parallel over the batch axis B=8 -> one sequence b per core.
Each core computes the GAT over its 16 graphs (t=0..15), global-max-pools,
runs the LSTM over its sequence locally, and emits one [8] output row.

GAT aggregation (per core, per graph g, head h). The per-dst softmax factor
v[n] = exp(a_d[n]) cancels between numerator and denominator, so with
  u[m] = exp(a_s[m]),  u5[m] = exp(0.2 a_s[m]),  w[n] = exp(-0.8 a_d[n])
the dense (src m, dst n) attention tile reduces to
  tS[m, n] = max(u[m], u5[m] * w[n])          (one DVE tensor_scalar, 2x mode)
  tA[m, n] = tS * cnt[m, n]                   (TT on DVE or gpsimd)
  out[n,f], den[n] via PE matmul with lhsT = [xp_h | ones] (33 rows out)
  gat[n,f] = relu(out/den + b_gat); pooled = max over n.

softmax max-subtraction is dropped: alpha = tA/sum(tA) is invariant to the
per-dst shift and exp of |a| <~ 10 cannot overflow bf16.

Engine budget per (g,h): PE 16 matmuls (oph), DVE 8 tensor_scalar + 5 TT +
epilogue, gpsimd wB/rb broadcasts + 3 TT, ACT copies/exp-fills/orel.
"""

import numpy as np

import concourse.bacc as bacc
import concourse.bass as bass
import concourse.mybir as mybir
import concourse.tile as tile
from concourse.bass_utils import run_bass_kernel_spmd

B, T, N, F_IN = 8, 16, 1000, 16
H, D = 4, 32
HD = H * D          # 128
HL = 64
OUT = 8
NEG = 0.2
NPAD = 1024         # padded node count
NBLK = 8            # src blocks of 128
G = T               # graphs per core
XW = 34             # xp33 stride per (J, h): 32 xp cols + ones col + pad

FP = mybir.dt.float32
BF = mybir.dt.bfloat16
AX = mybir.AxisListType
AF = mybir.ActivationFunctionType
OPS = mybir.AluOpType

_CACHE = {}


def _build_nc():
    nc = bacc.Bacc("TRN2", target_bir_lowering=False, debug=False)

    # ---- DRAM I/O ----
    d_xt = nc.dram_tensor("x_t", [F_IN, G * NPAD], BF, kind="ExternalInput").ap()
    d_cnt = nc.dram_tensor("cntmask", [128, NBLK * NPAD], BF, kind="ExternalInput").ap()
    d_wc = nc.dram_tensor("wc", [F_IN, 132], BF, kind="ExternalInput").ap()
    d_wa8 = nc.dram_tensor("wa8", [F_IN, 128], BF, kind="ExternalInput").ap()
    d_bgat = nc.dram_tensor("b_gat", [32, H], FP, kind="ExternalInput").ap()
    d_bb = nc.dram_tensor("blockones", [H, 128], BF, kind="ExternalInput").ap()
    d_ones1 = nc.dram_tensor("ones1", [1, 128], BF, kind="ExternalInput").ap()
    d_wih = nc.dram_tensor("wih_t", [HD, 4 * HL], FP, kind="ExternalInput").ap()
    d_whh = nc.dram_tensor("whh_t", [HL, 4 * HL], FP, kind="ExternalInput").ap()
    d_bls = nc.dram_tensor("b_lstm", [HL, 4], FP, kind="ExternalInput").ap()
    d_wclf = nc.dram_tensor("wclf_t", [HL, OUT], FP, kind="ExternalInput").ap()
    d_bclf = nc.dram_tensor("b_clf", [OUT, 1], FP, kind="ExternalInput").ap()
    d_y = nc.dram_tensor("y", [OUT, 1], FP, kind="ExternalOutput").ap()

    with tile.TileContext(nc) as tc:
        with (
            tc.tile_pool(name="const", bufs=1) as cpool,
            tc.tile_pool(name="stage", bufs=2) as spool,
            tc.tile_pool(name="wb", bufs=3) as wpool,
            tc.tile_pool(name="edense", bufs=3) as epool,
            tc.tile_pool(name="epi", bufs=3) as mpool,
            tc.tile_pool(name="lstm", bufs=2) as lpool,
        ):
            # ---- load constants ----
            c_xt = cpool.tile([F_IN, G * NPAD], BF, tag="xt")
            nc.sync.dma_start(c_xt[:], d_xt)
            c_cnt = cpool.tile([128, NBLK * NPAD], BF, tag="cnt")
            nc.sync.dma_start(c_cnt[:], d_cnt)
            c_wc = cpool.tile([F_IN, 132], BF, tag="wc")
            nc.sync.dma_start(c_wc[:], d_wc)
            c_wa8 = cpool.tile([F_IN, 128], BF, tag="wa8")
            nc.sync.dma_start(c_wa8[:], d_wa8)
            c_bgat = cpool.tile([32, H], FP, tag="bgat")
            nc.sync.dma_start(c_bgat[:], d_bgat)
            c_bb = cpool.tile([H, 128], BF, tag="bb")
            nc.sync.dma_start(c_bb[:], d_bb)
            c_ones1 = cpool.tile([1, 128], BF, tag="ones1")
            nc.sync.dma_start(c_ones1[:], d_ones1)
            c_wih = cpool.tile([HD, 4 * HL], FP, tag="wih")
            nc.sync.dma_start(c_wih[:], d_wih)
            c_whh = cpool.tile([HL, 4 * HL], FP, tag="whh")
            nc.sync.dma_start(c_whh[:], d_whh)
            c_bls = cpool.tile([HL, 4], FP, tag="bls")
            nc.sync.dma_start(c_bls[:], d_bls)
            c_wclf = cpool.tile([HL, OUT], FP, tag="wclf")
            nc.sync.dma_start(c_wclf[:], d_wclf)
            c_bclf = cpool.tile([OUT, 1], FP, tag="bclf")
            nc.sync.dma_start(c_bclf[:], d_bclf)

            c_pool = cpool.tile([HD, G], FP, tag="pooled")

            gat_pools = (
                tc.tile_pool(name="ps_stage", bufs=2, space="PSUM"),
                tc.tile_pool(name="ps_out", bufs=1, space="PSUM"),
                tc.tile_pool(name="ps_rb", bufs=1, space="PSUM"),
                tc.tile_pool(name="ps_wb", bufs=1, space="PSUM"),
                tc.tile_pool(name="ps_l", bufs=1, space="PSUM"),
            )
            ps_stage = gat_pools[0].__enter__()
            ps_out = gat_pools[1].__enter__()
            ps_rb = gat_pools[2].__enter__()
            ps_wb = gat_pools[3].__enter__()
            ps_l = gat_pools[4].__enter__()
            def issue_epilogue(eg, st):
                # batched per-graph epilogue: one reciprocal for all 4 heads,
                # PE block-broadcast of the reciprocal rows, then per-head
                # divide + bias + relu into porel.
                rec4 = mpool.tile([32, 128], BF, tag="rec4")
                with nc.allow_low_precision(reason="bf16 reciprocal: 0.4% rel err ok"):
                    nc.vector.reciprocal(rec4[:], st["den4"][:])
                rech4 = mpool.tile([H, NPAD], BF, tag="rech4")
                for h in range(H):
                    nc.sync.dma_start(
                        rech4[h:h + 1, :],
                        rec4[:].rearrange("i (h j) -> i h j", j=32)[:, h, :],
                    )
                rb = ps_rb.tile([128, NPAD], FP, tag="rb")
                for half in range(2):
                    nc.tensor.matmul(
                        rb[:, half * 512:(half + 1) * 512], c_bb[:],
                        rech4[:, half * 512:(half + 1) * 512],
                        start=True, stop=True,
                    )
                for h in range(H):
                    rbS = mpool.tile([32, NPAD], BF, tag=f"rbS{h}")
                    nc.scalar.copy(rbS[:], rb[32 * h:32 * (h + 1), :])
                    odiv = mpool.tile([32, NPAD], BF, tag="odiv")
                    nc.vector.tensor_tensor(
                        odiv[:], st["ophS"][h][0:32, :], rbS[:], OPS.mult,
                    )
                    nc.scalar.activation(
                        st["porel"][32 * h:32 * (h + 1), :], odiv[:], AF.Relu,
                        bias=c_bgat[:, h:h + 1], scale=1.0,
                    )

            def issue_reduce(eg, st):
                nc.vector.tensor_reduce(
                    c_pool[:, eg:eg + 1], st["porel"][:, 0:N], AX.X, OPS.max
                )

            # LSTM state: h stored as h2 = 2h (W_hh/W_clf pre-halved on
            # host); c stored as c2 = 2c (tanh applied with scale=0.5).
            lstm_h0 = lpool.tile([HL, 1], FP, tag="h0")
            lstm_c0 = lpool.tile([HL, 1], FP, tag="c0")
            lstate = {"h": lstm_h0, "c": lstm_c0}
            nc.vector.memset(lstate["h"][:], 0.0)
            nc.vector.memset(lstate["c"][:], 0.0)

            def issue_lstm_step(t):
                hprev, cprev = lstate["h"], lstate["c"]
                tga = []
                for gate in range(4):
                    psgt = ps_l.tile([HL, 4], FP, tag="psg")
                    psg = psgt[:, gate:gate + 1]
                    nc.tensor.matmul(
                        psg, c_wih[:, gate * HL:(gate + 1) * HL],
                        c_pool[:, t:t + 1], start=True, stop=False,
                    )
                    nc.tensor.matmul(
                        psg, c_whh[:, gate * HL:(gate + 1) * HL],
                        hprev[:], start=False, stop=True,
                    )
                    tgt = lpool.tile([HL, 1], FP, tag=f"tg{gate}")
                    # gates i,f,o: sigmoid via tanh-half; gate g: plain tanh
                    sc = 1.0 if gate == 2 else 0.5
                    nc.scalar.activation(
                        tgt[:], psg, AF.Tanh,
                        bias=c_bls[:, gate:gate + 1], scale=sc,
                    )
                    tga.append(tgt)
                ti, tf, tg_, to = tga
                # v1 = (tf+1)*c2 = 4*sig(f)*c ; v2 = (ti+1)*tg = 2*sig(i)*g
                v1 = lpool.tile([HL, 1], FP, tag="v1")
                nc.vector.scalar_tensor_tensor(
                    v1[:], tf[:], 1.0, cprev[:], OPS.add, OPS.mult
                )
                v2 = lpool.tile([HL, 1], FP, tag="v2")
                nc.vector.scalar_tensor_tensor(
                    v2[:], ti[:], 1.0, tg_[:], OPS.add, OPS.mult
                )
                cnew = lpool.tile([HL, 1], FP, tag="c0")
                nc.vector.scalar_tensor_tensor(
                    cnew[:], v1[:], 0.5, v2[:], OPS.mult, OPS.add
                )
                tcn = lpool.tile([HL, 1], FP, tag="tcn")
                nc.scalar.activation(tcn[:], cnew[:], AF.Tanh, scale=0.5)
                hnew = lpool.tile([HL, 1], FP, tag="h0")
                nc.vector.scalar_tensor_tensor(
                    hnew[:], to[:], 1.0, tcn[:], OPS.add, OPS.mult
                )
                lstate["h"], lstate["c"] = hnew, cnew

            prev = None
            for g in range(G):
                goff = g * NPAD
                # ---- stage: per-J combined matmul -> xp (m-partitioned) + a_s
                xp33 = spool.tile([128, NBLK * H * XW], BF, tag="xp33")
                nc.vector.memset(
                    xp33[:].rearrange("p (J h q) -> p J h q", h=H, q=XW)[
                        :, :, :, 32:33
                    ],
                    1.0,
                )
                u_t = spool.tile([128, NBLK * H], FP, tag="u_t")
                u5_t = spool.tile([128, NBLK * H], FP, tag="u5_t")
                for J in range(NBLK):
                    pS = ps_stage.tile([128, 512], FP, tag="st")
                    pS = pS[:, 0:132]
                    nc.tensor.matmul(
                        pS[:], c_xt[:, goff + J * 128:goff + (J + 1) * 128],
                        c_wc[:], start=True, stop=True,
                    )
                    # xp -> xp33 strided slots (ACT copy, bf16)
                    nc.scalar.copy(
                        xp33[:, J * H * XW:(J + 1) * H * XW].rearrange(
                            "p (h q) -> p h q", q=XW
                        )[:, :, 0:32],
                        pS[:, 0:128].rearrange("p (h q) -> p h q", q=32),
                    )
                    nc.scalar.activation(
                        u_t[:, J * H:(J + 1) * H], pS[:, 128:132], AF.Exp,
                        scale=1.0,
                    )
                    nc.scalar.activation(
                        u5_t[:, J * H:(J + 1) * H], pS[:, 128:132], AF.Exp,
                        scale=NEG,
                    )
                # w rows: exp(-0.8 a_d) at partitions 32h of s8w
                s8w = spool.tile([128, NPAD], BF, tag="s8w")
                for half in range(2):
                    pw = ps_stage.tile([128, 512], FP, tag="st")
                    nc.tensor.matmul(
                        pw[:], c_wa8[:],
                        c_xt[:, goff + half * 512:goff + (half + 1) * 512],
                        start=True, stop=True,
                    )
                    nc.scalar.activation(
                        s8w[:, half * 512:(half + 1) * 512], pw[:], AF.Exp,
                        scale=NEG - 1.0,
                    )

                porel = spool.tile([128, NPAD], BF, tag="porel")
                den4 = mpool.tile([32, 128], BF, tag="den4")
                cur = {"den4": den4, "porel": porel, "ophS": []}

                # partition_broadcast only reads physical partition 0: relocate
                # all four w rows there at stage time (keeps these DMAs clear
                # of the den4-fold DMAs in the queue)
                wrs = []
                for h in range(H):
                    wr = wpool.tile([1, NPAD], BF, tag=f"wrow{h}")
                    nc.sync.dma_start(wr[:], s8w[32 * h:32 * h + 1, :])
                    wrs.append(wr)

                def issue_wb(h):
                    # broadcast w row to 128 partitions: PE ones-matmul into
                    # PSUM (one bank, 2 halves), ACT copy each to SBUF bf16
                    wB = wpool.tile([128, NPAD], BF, tag="wB")
                    for half in range(2):
                        wbp = ps_wb.tile([128, 512], FP, tag="wbp")
                        nc.tensor.matmul(
                            wbp[:], c_ones1[:],
                            wrs[h][:, half * 512:(half + 1) * 512],
                            start=True, stop=True,
                        )
                        nc.scalar.copy(
                            wB[:, half * 512:(half + 1) * 512], wbp[:]
                        )
                    return wB

                wbs = [issue_wb(0)]
                for h in range(H):
                    if h == 1 and prev is not None:
                        issue_epilogue(g - 1, prev)
                    if h == 2 and prev is not None:
                        issue_reduce(g - 1, prev)
                        issue_lstm_step(g - 1)
                    if h < H - 1:
                        wbs.append(issue_wb(h + 1))
                    wB = wbs[h]
                    if h % 2 == 0:
                        ophDB = ps_out.tile([128, NPAD], FP, tag="oph")
                    oph = ophDB[64 * (h % 2):64 * (h % 2) + 33]
                    for J in range(NBLK):
                        col = J * H + h
                        tS = epool.tile([128, NPAD], BF, tag="tS")
                        nc.vector.tensor_scalar(
                            tS[:], wB[:], u5_t[:, col:col + 1],
                            u_t[:, col:col + 1], OPS.mult, OPS.max,
                        )
                        tA = epool.tile([128, NPAD], BF, tag="tA")
                        eng = nc.vector
                        eng.tensor_tensor(
                            tA[:], tS[:], c_cnt[:, J * NPAD:(J + 1) * NPAD],
                            OPS.mult,
                        )
                        base = J * H * XW + h * XW
                        for half in range(2):
                            nc.tensor.matmul(
                                oph[:, half * 512:(half + 1) * 512],
                                xp33[:, base:base + 33],
                                tA[:, half * 512:(half + 1) * 512],
                                start=(J == 0), stop=(J == NBLK - 1),
                            )
                    # ---- per-head: copy PSUM out + stash den row ----
                    ophS = mpool.tile([33, NPAD], BF, tag=f"ophS{h}")
                    nc.scalar.copy(ophS[:], oph[:])
                    nc.sync.dma_start(
                        den4[:, 32 * h:32 * (h + 1)], ophS[32:33, :]
                    )
                    cur["ophS"].append(ophS)
                prev = cur
            issue_epilogue(G - 1, prev)
            issue_reduce(G - 1, prev)
            issue_lstm_step(G - 1)

            ps3t = ps_l.tile([HL, 4], FP, tag="psg")
            ps3 = ps3t[0:OUT, 0:1]
            nc.tensor.matmul(ps3, c_wclf[:], lstate["h"][:], start=True, stop=True)
            ysb = lpool.tile([OUT, 1], FP, tag="ysb")
            nc.vector.tensor_tensor(ysb[:], ps3, c_bclf[:], OPS.add)
            nc.sync.dma_start(d_y, ysb[:])

            gat_pools[4].__exit__(None, None, None)
            gat_pools[3].__exit__(None, None, None)
            gat_pools[2].__exit__(None, None, None)
            gat_pools[1].__exit__(None, None, None)
            gat_pools[0].__exit__(None, None, None)

    nc.compile()
    return nc


def _host_prep(inputs):
    x = np.asarray(inputs["x"], dtype=np.float32)          # [B, T, N, F]
    ei = np.asarray(inputs["edge_index"])
    W_gat = np.asarray(inputs["W_gat"], dtype=np.float32)  # [16, 128]
    att_src = np.asarray(inputs["att_src"], dtype=np.float32)  # [H, D]
    att_dst = np.asarray(inputs["att_dst"], dtype=np.float32)
    b_gat = np.asarray(inputs["b_gat"], dtype=np.float32)
    W_ih = np.asarray(inputs["W_ih"], dtype=np.float32)    # [256, 128]
    W_hh = np.asarray(inputs["W_hh"], dtype=np.float32)    # [256, 64]
    b_ih = np.asarray(inputs["b_ih"], dtype=np.float32)
    b_hh = np.asarray(inputs["b_hh"], dtype=np.float32)
    W_clf = np.asarray(inputs["W_clf"], dtype=np.float32)  # [8, 64]
    b_clf = np.asarray(inputs["b_clf"], dtype=np.float32)

    bf16 = mybir.dt.np(BF)

    # fold attention vectors: a_s = x @ (W_gat-reshaped @ att_src)
    Wr = W_gat.reshape(F_IN, H, D)
    W_as = np.einsum("fhd,hd->fh", Wr, att_src)            # [16, 4]
    W_ad = np.einsum("fhd,hd->fh", Wr, att_dst)
    wc = np.zeros((F_IN, 132), dtype=np.float32)
    wc[:, 0:128] = W_gat
    wc[:, 128:132] = W_as
    wa8 = np.zeros((F_IN, 128), dtype=np.float32)
    wa8[:, 32 * np.arange(H)] = W_ad                       # a_d -> partition 32h

    # edge counts with self loops, dense [1024, 1024]
    src = ei[0].astype(np.int64)
    dst = ei[1].astype(np.int64)
    Cm = np.zeros((NPAD, NPAD), dtype=np.float32)
    np.add.at(Cm, (src, dst), 1.0)
    Cm[np.arange(N), np.arange(N)] += 1.0                  # self loops
    Cm[NPAD - 1, N:] = 1.0  # dummy edges: keep pad-column denominators finite
    cntmask = (
        Cm.reshape(NBLK, 128, NPAD).transpose(1, 0, 2).reshape(128, NBLK * NPAD)
    ).astype(bf16)

    # x transposed per core: [F, T*NPAD] bf16
    xpad = np.zeros((B, T, NPAD, F_IN), dtype=np.float32)
    xpad[:, :, :N, :] = x
    xts = [
        np.ascontiguousarray(xpad[b].reshape(T * NPAD, F_IN).T).astype(bf16)
        for b in range(B)
    ]

    b_gates = (b_ih + b_hh).astype(np.float32)             # [256]
    bls = np.zeros((HL, 4), dtype=np.float32)
    bls[:, 0] = 0.5 * b_gates[0:64]                        # i (tanh-half trick)
    bls[:, 1] = 0.5 * b_gates[64:128]                      # f
    bls[:, 2] = b_gates[128:192]                           # g
    bls[:, 3] = 0.5 * b_gates[192:256]                     # o

    bb = np.zeros((H, 128), dtype=np.float32)
    for h in range(H):
        bb[h, 32 * h:32 * (h + 1)] = 1.0

    common = {
        "blockones": bb.astype(bf16),
        "ones1": np.ones((1, 128), dtype=bf16),
        "cntmask": cntmask,
        "wc": wc.astype(bf16),
        "wa8": wa8.astype(bf16),
        "b_gat": np.ascontiguousarray(b_gat.reshape(H, 32).T),
        "wih_t": np.ascontiguousarray(W_ih.T),             # [128, 256]
        "whh_t": np.ascontiguousarray(0.5 * W_hh.T),       # [64, 256] (h2 comp)
        "b_lstm": bls,
        "wclf_t": np.ascontiguousarray(0.5 * W_clf.T),     # [64, 8] (h2 comp)
        "b_clf": b_clf.reshape(OUT, 1),
    }
    in_maps = []
    for b in range(B):
        m = dict(common)
        m["x_t"] = xts[b]
        in_maps.append(m)
    return in_maps


def kernel(**inputs):
    if "nc" not in _CACHE:
        _CACHE["nc"] = _build_nc()
    nc = _CACHE["nc"]
    in_maps = _host_prep(inputs)
    res = run_bass_kernel_spmd(nc, in_maps, core_ids=list(range(B)))
    y = np.stack([r["y"][:, 0] for r in res.results], axis=0)
    return y.astype(np.float32)


if __name__ == "__main__":
    import reference as R

    inp = R.setup_inputs()
    inp = {k: np.asarray(v) for k, v in inp.items()}
    out = kernel(**inp)
    print(out)


# revision 25
# speedup vs baseline: 1.1493x; 1.0309x over previous
"""GAT + global-max-pool + LSTM + Linear kernel for Trainium2 (8 NeuronCores).

Sharding: data-parallel over the batch axis B=8 -> one sequence b per core.
Each core computes the GAT over its 16 graphs (t=0..15), global-max-pools,
runs the LSTM over its sequence locally, and emits one [8] output row.

GAT aggregation (per core, per graph g, head h). The per-dst softmax factor
v[n] = exp(a_d[n]) cancels between numerator and denominator, so with
  u[m] = exp(a_s[m]),  u5[m] = exp(0.2 a_s[m]),  w[n] = exp(-0.8 a_d[n])
the dense (src m, dst n) attention tile reduces to
  tS[m, n] = max(u[m], u5[m] * w[n])          (one DVE tensor_scalar, 2x mode)
  tA[m, n] = tS * cnt[m, n]                   (TT on DVE or gpsimd)
  out[n,f], den[n] via PE matmul with lhsT = [xp_h | ones] (33 rows out)
  gat[n,f] = relu(out/den + b_gat); pooled = max over n.

softmax max-subtraction is dropped: alpha = tA/sum(tA) is invariant to the
per-dst shift and exp of |a| <~ 10 cannot overflow bf16.

Engine budget per (g,h): PE 16 matmuls (oph), DVE 8 tensor_scalar + 5 TT +
epilogue, gpsimd wB/rb broadcasts + 3 TT, ACT copies/exp-fills/orel.
"""

import numpy as np

import concourse.bacc as bacc
import concourse.bass as bass
import concourse.mybir as mybir
import concourse.tile as tile
from concourse.bass_utils import run_bass_kernel_spmd

B, T, N, F_IN = 8, 16, 1000, 16
H, D = 4, 32
HD = H * D          # 128
HL = 64
OUT = 8
NEG = 0.2
NPAD = 1024         # padded node count
NBLK = 8            # src blocks of 128
G = T               # graphs per core
XW = 34             # xp33 stride per (J, h): 32 xp cols + ones col + pad

FP = mybir.dt.float32
BF = mybir.dt.bfloat16
AX = mybir.AxisListType
AF = mybir.ActivationFunctionType
OPS = mybir.AluOpType

_CACHE = {}


def _build_nc():
    nc = bacc.Bacc("TRN2", target_bir_lowering=False, debug=False)

    # ---- DRAM I/O ----
    d_xt = nc.dram_tensor("x_t", [F_IN, G * NPAD], BF, kind="ExternalInput").ap()
    d_cnt = nc.dram_tensor("cntmask", [128, NBLK * NPAD], BF, kind="ExternalInput").ap()
    d_wc = nc.dram_tensor("wc", [F_IN, 132], BF, kind="ExternalInput").ap()
    d_wa8 = nc.dram_tensor("wa8", [F_IN, 128], BF, kind="ExternalInput").ap()
    d_bgat = nc.dram_tensor("b_gat", [32, H], FP, kind="ExternalInput").ap()
    d_bb = nc.dram_tensor("blockones", [H, 128], BF, kind="ExternalInput").ap()
    d_ones1 = nc.dram_tensor("ones1", [1, 128], BF, kind="ExternalInput").ap()
    d_wih = nc.dram_tensor("wih_t", [HD, 4 * HL], FP, kind="ExternalInput").ap()
    d_whh = nc.dram_tensor("whh_t", [HL, 4 * HL], FP, kind="ExternalInput").ap()
    d_bls = nc.dram_tensor("b_lstm", [HL, 4], FP, kind="ExternalInput").ap()
    d_wclf = nc.dram_tensor("wclf_t", [HL, OUT], FP, kind="ExternalInput").ap()
    d_bclf = nc.dram_tensor("b_clf", [OUT, 1], FP, kind="ExternalInput").ap()
    d_y = nc.dram_tensor("y", [OUT, 1], FP, kind="ExternalOutput").ap()

    with tile.TileContext(nc) as tc:
        with (
            tc.tile_pool(name="const", bufs=1) as cpool,
            tc.tile_pool(name="stage", bufs=2) as spool,
            tc.tile_pool(name="wb", bufs=3) as wpool,
            tc.tile_pool(name="edense", bufs=3) as epool,
            tc.tile_pool(name="epi", bufs=3) as mpool,
            tc.tile_pool(name="lstm", bufs=2) as lpool,
        ):
            # ---- load constants ----
            c_xt = cpool.tile([F_IN, G * NPAD], BF, tag="xt")
            nc.sync.dma_start(c_xt[:], d_xt)
            c_wc = cpool.tile([F_IN, 132], BF, tag="wc")
            nc.sync.dma_start(c_wc[:], d_wc)
            c_wa8 = cpool.tile([F_IN, 128], BF, tag="wa8")
            nc.sync.dma_start(c_wa8[:], d_wa8)
            c_cnt = cpool.tile([128, NBLK * NPAD], BF, tag="cnt")
            nc.sync.dma_start(c_cnt[:], d_cnt)
            c_bgat = cpool.tile([32, H], FP, tag="bgat")
            nc.sync.dma_start(c_bgat[:], d_bgat)
            c_bb = cpool.tile([H, 128], BF, tag="bb")
            nc.sync.dma_start(c_bb[:], d_bb)
            c_ones1 = cpool.tile([1, 128], BF, tag="ones1")
            nc.sync.dma_start(c_ones1[:], d_ones1)
            c_wih = cpool.tile([HD, 4 * HL], FP, tag="wih")
            nc.sync.dma_start(c_wih[:], d_wih)
            c_whh = cpool.tile([HL, 4 * HL], FP, tag="whh")
            nc.sync.dma_start(c_whh[:], d_whh)
            c_bls = cpool.tile([HL, 4], FP, tag="bls")
            nc.sync.dma_start(c_bls[:], d_bls)
            c_wclf = cpool.tile([HL, OUT], FP, tag="wclf")
            nc.sync.dma_start(c_wclf[:], d_wclf)
            c_bclf = cpool.tile([OUT, 1], FP, tag="bclf")
            nc.sync.dma_start(c_bclf[:], d_bclf)

            c_pool = cpool.tile([HD, G], FP, tag="pooled")

            gat_pools = (
                tc.tile_pool(name="ps_stage", bufs=2, space="PSUM"),
                tc.tile_pool(name="ps_out", bufs=1, space="PSUM"),
                tc.tile_pool(name="ps_rb", bufs=1, space="PSUM"),
                tc.tile_pool(name="ps_wb", bufs=1, space="PSUM"),
            )
            ps_stage = gat_pools[0].__enter__()
            ps_out = gat_pools[1].__enter__()
            ps_rb = gat_pools[2].__enter__()
            ps_wb = gat_pools[3].__enter__()
            def issue_epilogue(eg, st):
                # batched per-graph epilogue: one reciprocal for all 4 heads,
                # PE block-broadcast of the reciprocal rows, then per-head
                # divide + bias + relu into porel.
                rec4 = mpool.tile([32, 128], BF, tag="rec4")
                with nc.allow_low_precision(reason="bf16 reciprocal: 0.4% rel err ok"):
                    nc.vector.reciprocal(rec4[:], st["den4"][:])
                rech4 = mpool.tile([H, NPAD], BF, tag="rech4")
                for h in range(H):
                    nc.sync.dma_start(
                        rech4[h:h + 1, :],
                        rec4[:].rearrange("i (h j) -> i h j", j=32)[:, h, :],
                    )
                rb = ps_rb.tile([128, NPAD], FP, tag="rb")
                for half in range(2):
                    nc.tensor.matmul(
                        rb[:, half * 512:(half + 1) * 512], c_bb[:],
                        rech4[:, half * 512:(half + 1) * 512],
                        start=True, stop=True,
                    )
                for h in range(H):
                    rbS = mpool.tile([32, NPAD], BF, tag=f"rbS{h}")
                    nc.scalar.copy(rbS[:, 0:N], rb[32 * h:32 * (h + 1), 0:N])
                    odiv = mpool.tile([32, NPAD], BF, tag="odiv")
                    nc.vector.tensor_tensor(
                        odiv[:, 0:N], st["ophS"][h][0:32, 0:N], rbS[:, 0:N],
                        OPS.mult,
                    )
                    nc.scalar.activation(
                        st["porel"][32 * h:32 * (h + 1), 0:N], odiv[:, 0:N],
                        AF.Relu, bias=c_bgat[:, h:h + 1], scale=1.0,
                    )

            def issue_reduce(eg, st):
                nc.vector.tensor_reduce(
                    c_pool[:, eg:eg + 1], st["porel"][:, 0:N], AX.X, OPS.max
                )

            prev = None
            for g in range(G):
                goff = g * NPAD
                # ---- stage: per-J combined matmul -> xp (m-partitioned) + a_s
                xp33 = spool.tile([128, NBLK * H * XW], BF, tag="xp33")
                nc.vector.memset(
                    xp33[:].rearrange("p (J h q) -> p J h q", h=H, q=XW)[
                        :, :, :, 32:33
                    ],
                    1.0,
                )
                u_t = spool.tile([128, NBLK * H], FP, tag="u_t")
                u5_t = spool.tile([128, NBLK * H], FP, tag="u5_t")
                for J in range(NBLK):
                    pS = ps_stage.tile([128, 512], FP, tag="st")
                    pS = pS[:, 0:132]
                    nc.tensor.matmul(
                        pS[:], c_xt[:, goff + J * 128:goff + (J + 1) * 128],
                        c_wc[:], start=True, stop=True,
                    )
                    # xp -> xp33 strided slots (ACT copy, bf16)
                    nc.scalar.copy(
                        xp33[:, J * H * XW:(J + 1) * H * XW].rearrange(
                            "p (h q) -> p h q", q=XW
                        )[:, :, 0:32],
                        pS[:, 0:128].rearrange("p (h q) -> p h q", q=32),
                    )
                    nc.scalar.activation(
                        u_t[:, J * H:(J + 1) * H], pS[:, 128:132], AF.Exp,
                        scale=1.0,
                    )
                    nc.scalar.activation(
                        u5_t[:, J * H:(J + 1) * H], pS[:, 128:132], AF.Exp,
                        scale=NEG,
                    )
                # w rows: exp(-0.8 a_d) at partitions 32h of s8w
                s8w = spool.tile([128, NPAD], BF, tag="s8w")
                for half in range(2):
                    pw = ps_stage.tile([128, 512], FP, tag="st")
                    nc.tensor.matmul(
                        pw[:], c_wa8[:],
                        c_xt[:, goff + half * 512:goff + (half + 1) * 512],
                        start=True, stop=True,
                    )
                    nc.scalar.activation(
                        s8w[:, half * 512:(half + 1) * 512], pw[:], AF.Exp,
                        scale=NEG - 1.0,
                    )

                porel = spool.tile([128, NPAD], BF, tag="porel")
                den4 = mpool.tile([32, 128], BF, tag="den4")
                cur = {"den4": den4, "porel": porel, "ophS": []}

                # partition_broadcast only reads physical partition 0: relocate
                # all four w rows there at stage time (keeps these DMAs clear
                # of the den4-fold DMAs in the queue)
                wrs = []
                for h in range(H):
                    wr = wpool.tile([1, NPAD], BF, tag=f"wrow{h}")
                    nc.sync.dma_start(wr[:], s8w[32 * h:32 * h + 1, :])
                    wrs.append(wr)

                def issue_wb(h):
                    # broadcast w row to 128 partitions: PE ones-matmul into
                    # PSUM, then ACT copy to SBUF bf16 (gpsimd stays pure-TT:
                    # op-type switches cost ~6us reconfig)
                    wbp = ps_wb.tile([128, NPAD], FP, tag="wbp")
                    for half in range(2):
                        nc.tensor.matmul(
                            wbp[:, half * 512:(half + 1) * 512], c_ones1[:],
                            wrs[h][:, half * 512:(half + 1) * 512],
                            start=True, stop=True,
                        )
                    wB = wpool.tile([128, NPAD], BF, tag="wB")
                    nc.scalar.copy(wB[:], wbp[:])
                    return wB

                wbs = [issue_wb(0)]
                for h in range(H):
                    if h == 1 and prev is not None:
                        issue_epilogue(g - 1, prev)
                    if h == 2 and prev is not None:
                        issue_reduce(g - 1, prev)
                    if h < H - 1:
                        wbs.append(issue_wb(h + 1))
                    wB = wbs[h]
                    if h % 2 == 0:
                        ophDB = ps_out.tile([128, NPAD], FP, tag="oph")
                    oph = ophDB[64 * (h % 2):64 * (h % 2) + 33]
                    for J in range(NBLK):
                        col = J * H + h
                        tS = epool.tile([128, NPAD], BF, tag="tS")
                        nc.vector.tensor_scalar(
                            tS[:, 0:N], wB[:, 0:N], u5_t[:, col:col + 1],
                            u_t[:, col:col + 1], OPS.mult, OPS.max,
                        )
                        tA = epool.tile([128, NPAD], BF, tag="tA")
                        nc.vector.tensor_tensor(
                            tA[:, 0:N], tS[:, 0:N],
                            c_cnt[:, J * NPAD:J * NPAD + N], OPS.mult,
                        )
                        base = J * H * XW + h * XW
                        for half in range(2):
                            hi = 512 if half == 0 else N - 512
                            nc.tensor.matmul(
                                oph[:, half * 512:half * 512 + hi],
                                xp33[:, base:base + 33],
                                tA[:, half * 512:half * 512 + hi],
                                start=(J == 0), stop=(J == NBLK - 1),
                            )
                    # ---- per-head: copy PSUM out + stash den row ----
                    ophS = mpool.tile([33, NPAD], BF, tag=f"ophS{h}")
                    nc.vector.memset(ophS[:, N:], 1.0)
                    nc.scalar.copy(ophS[:, 0:N], oph[:, 0:N])
                    nc.sync.dma_start(
                        den4[:, 32 * h:32 * (h + 1)], ophS[32:33, :]
                    )
                    cur["ophS"].append(ophS)
                prev = cur
            issue_epilogue(G - 1, prev)
            issue_reduce(G - 1, prev)

            gat_pools[3].__exit__(None, None, None)
            gat_pools[2].__exit__(None, None, None)
            gat_pools[1].__exit__(None, None, None)
            gat_pools[0].__exit__(None, None, None)
            lstm_pool_cm = tc.tile_pool(name="ps_lstm", bufs=2, space="PSUM")
            ps_lstm = lstm_pool_cm.__enter__()
            # ---- LSTM over T steps ----
            # h is stored as h2 = 2h (W_hh/W_clf pre-halved on host);
            # c is stored as c2 = 2c (tanh applied with scale=0.5).
            hprev = lpool.tile([HL, 1], FP, tag="h0")
            cprev = lpool.tile([HL, 1], FP, tag="c0")
            nc.vector.memset(hprev[:], 0.0)
            nc.vector.memset(cprev[:], 0.0)
            for t in range(T):
                tga = []
                for gate in range(4):
                    psg = ps_lstm.tile([HL, 1], FP, tag="psg")
                    nc.tensor.matmul(
                        psg[:], c_wih[:, gate * HL:(gate + 1) * HL],
                        c_pool[:, t:t + 1], start=True, stop=False,
                    )
                    nc.tensor.matmul(
                        psg[:], c_whh[:, gate * HL:(gate + 1) * HL],
                        hprev[:], start=False, stop=True,
                    )
                    tgt = lpool.tile([HL, 1], FP, tag=f"tg{gate}")
                    # gates i,f,o: sigmoid via tanh-half; gate g: plain tanh
                    sc = 1.0 if gate == 2 else 0.5
                    nc.scalar.activation(
                        tgt[:], psg[:], AF.Tanh,
                        bias=c_bls[:, gate:gate + 1], scale=sc,
                    )
                    tga.append(tgt)
                ti, tf, tg_, to = tga
                # v1 = (tf+1)*c2 = 4*sig(f)*c ; v2 = (ti+1)*tg = 2*sig(i)*g
                # c2_new = 2c_new = v1/2 + v2
                v1 = lpool.tile([HL, 1], FP, tag="v1")
                nc.vector.scalar_tensor_tensor(
                    v1[:], tf[:], 1.0, cprev[:], OPS.add, OPS.mult
                )
                v2 = lpool.tile([HL, 1], FP, tag="v2")
                nc.vector.scalar_tensor_tensor(
                    v2[:], ti[:], 1.0, tg_[:], OPS.add, OPS.mult
                )
                cnew = lpool.tile([HL, 1], FP, tag="c0")
                nc.vector.scalar_tensor_tensor(
                    cnew[:], v1[:], 0.5, v2[:], OPS.mult, OPS.add
                )
                tcn = lpool.tile([HL, 1], FP, tag="tcn")
                nc.scalar.activation(tcn[:], cnew[:], AF.Tanh, scale=0.5)
                hnew = lpool.tile([HL, 1], FP, tag="h0")
                # h2 = (to + 1) * tanh(c)
                nc.vector.scalar_tensor_tensor(
                    hnew[:], to[:], 1.0, tcn[:], OPS.add, OPS.mult
                )
                hprev, cprev = hnew, cnew

            ps3 = ps_lstm.tile([OUT, 1], FP, tag="ps3")
            nc.tensor.matmul(ps3[:], c_wclf[:], hprev[:], start=True, stop=True)
            ysb = lpool.tile([OUT, 1], FP, tag="ysb")
            nc.vector.tensor_tensor(ysb[:], ps3[:], c_bclf[:], OPS.add)
            nc.sync.dma_start(d_y, ysb[:])
            lstm_pool_cm.__exit__(None, None, None)

    nc.compile()
    return nc


def _host_prep(inputs):
    x = np.asarray(inputs["x"], dtype=np.float32)          # [B, T, N, F]
    ei = np.asarray(inputs["edge_index"])
    W_gat = np.asarray(inputs["W_gat"], dtype=np.float32)  # [16, 128]
    att_src = np.asarray(inputs["att_src"], dtype=np.float32)  # [H, D]
    att_dst = np.asarray(inputs["att_dst"], dtype=np.float32)
    b_gat = np.asarray(inputs["b_gat"], dtype=np.float32)
    W_ih = np.asarray(inputs["W_ih"], dtype=np.float32)    # [256, 128]
    W_hh = np.asarray(inputs["W_hh"], dtype=np.float32)    # [256, 64]
    b_ih = np.asarray(inputs["b_ih"], dtype=np.float32)
    b_hh = np.asarray(inputs["b_hh"], dtype=np.float32)
    W_clf = np.asarray(inputs["W_clf"], dtype=np.float32)  # [8, 64]
    b_clf = np.asarray(inputs["b_clf"], dtype=np.float32)

    bf16 = mybir.dt.np(BF)

    # fold attention vectors: a_s = x @ (W_gat-reshaped @ att_src)
    Wr = W_gat.reshape(F_IN, H, D)
    W_as = np.einsum("fhd,hd->fh", Wr, att_src)            # [16, 4]
    W_ad = np.einsum("fhd,hd->fh", Wr, att_dst)
    wc = np.zeros((F_IN, 132), dtype=np.float32)
    wc[:, 0:128] = W_gat
    wc[:, 128:132] = W_as
    wa8 = np.zeros((F_IN, 128), dtype=np.float32)
    wa8[:, 32 * np.arange(H)] = W_ad                       # a_d -> partition 32h

    # edge counts with self loops, dense [1024, 1024]
    src = ei[0].astype(np.int64)
    dst = ei[1].astype(np.int64)
    Cm = np.zeros((NPAD, NPAD), dtype=np.float32)
    np.add.at(Cm, (src, dst), 1.0)
    Cm[np.arange(N), np.arange(N)] += 1.0                  # self loops
    Cm[NPAD - 1, N:] = 1.0  # dummy edges: keep pad-column denominators finite
    cntmask = (
        Cm.reshape(NBLK, 128, NPAD).transpose(1, 0, 2).reshape(128, NBLK * NPAD)
    ).astype(bf16)

    # x transposed per core: [F, T*NPAD] bf16
    xpad = np.zeros((B, T, NPAD, F_IN), dtype=np.float32)
    xpad[:, :, :N, :] = x
    xts = [
        np.ascontiguousarray(xpad[b].reshape(T * NPAD, F_IN).T).astype(bf16)
        for b in range(B)
    ]

    b_gates = (b_ih + b_hh).astype(np.float32)             # [256]
    bls = np.zeros((HL, 4), dtype=np.float32)
    bls[:, 0] = 0.5 * b_gates[0:64]                        # i (tanh-half trick)
    bls[:, 1] = 0.5 * b_gates[64:128]                      # f
    bls[:, 2] = b_gates[128:192]                           # g
    bls[:, 3] = 0.5 * b_gates[192:256]                     # o

    bb = np.zeros((H, 128), dtype=np.float32)
    for h in range(H):
        bb[h, 32 * h:32 * (h + 1)] = 1.0

    common = {
        "blockones": bb.astype(bf16),
        "ones1": np.ones((1, 128), dtype=bf16),
        "cntmask": cntmask,
        "wc": wc.astype(bf16),
        "wa8": wa8.astype(bf16),
        "b_gat": np.ascontiguousarray(b_gat.reshape(H, 32).T),
        "wih_t": np.ascontiguousarray(W_ih.T),             # [128, 256]
        "whh_t": np.ascontiguousarray(0.5 * W_hh.T),       # [64, 256] (h2 comp)
        "b_lstm": bls,
        "wclf_t": np.ascontiguousarray(0.5 * W_clf.T),     # [64, 8] (h2 comp)
        "b_clf": b_clf.reshape(OUT, 1),
    }
    in_maps = []
    for b in range(B):
        m = dict(common)
        m["x_t"] = xts[b]
        in_maps.append(m)
    return in_maps


def kernel(**inputs):
    if "nc" not in _CACHE:
        _CACHE["nc"] = _build_nc()
    nc = _CACHE["nc"]
    in_maps = _host_prep(inputs)
    res = run_bass_kernel_spmd(nc, in_maps, core_ids=list(range(B)))
    y = np.stack([r["y"][:, 0] for r in res.results], axis=0)
    return y.astype(np.float32)


if __name__ == "__main__":
    import reference as R

    inp = R.setup_inputs()
    inp = {k: np.asarray(v) for k, v in inp.items()}
    out = kernel(**inp)
    print(out)
